# revision 47
# baseline (speedup 1.0000x reference)
"""Multi-head attention (B=4, T=2048, dim=1024, 16 heads) on 8 NeuronCores.

Sharding: core c -> batch b = c//2, head-group g = c%2 (8 heads per core).
Each core computes QKV projection for its heads, full attention over its
8 heads, and a partial output projection (contraction over its 512 head
dims). Host sums the two fp16 partials per batch and adds b_out.

v2 design (fp16 everywhere, ACT/PE co-scheduled):
  All matmul operands fp16 (PE streams 1 col/cycle, same as f32r, but
  SBUF halves so the full T=2048 stays resident; PSUM accumulation fp32).
  Attention uses a "flipped" PV matmul: P^T tiles ([s,t-chunk]) are the
  stationary operand against the moving V_aug [s, 65] (65th col = ones
  for the softmax denominator), producing out_aug [t, 65] -- 520 streamed
  columns per (p,tb,st) instead of 1024.  Normalization is then a cheap
  per-partition broadcast (DVE tensor_scalar with an AP scalar), and the
  [t,e] -> [e,t] layout fix rides the XBAR DMA-transpose (free wrt PE).

  The single instruction stream interleaves score production (which feeds
  the serially-bound Activation engine: 256 exp tiles x ~1.04us = 266us)
  with V-projection / next-pair QK-projection / output-projection filler
  so the PE never idles: PV lags scores by 16 steps (pt pool bufs=20)
  which also lets the V projection stream in during pair 0's window.

  PSUM budget (8 banks): pss [128,1024]x2 (4) + proj [128,512]x2 (2) +
  augA/augB [128,260]x1 (2).
"""

import numpy as np

B, T, C = 4, 2048, 1024
HLOC = 8          # heads per core
D = 64
E = HLOC * D      # 512: local head-dim width
NCORES = 8

_CACHE = {}


def _build(reps=1):
    import concourse.mybir as mybir
    import concourse.tile as tile
    import concourse.bacc as bacc

    f32 = mybir.dt.float32
    f16 = mybir.dt.float16
    EXP = mybir.ActivationFunctionType.Exp

    nc = bacc.Bacc("TRN2", target_bir_lowering=False, debug=False)

    xT = nc.dram_tensor("xT", [C, T], f16, kind="ExternalInput").ap()
    wqkT = nc.dram_tensor("wqkT", [C, 2 * E], f16, kind="ExternalInput").ap()
    wvT = nc.dram_tensor("wvT", [C, E], f16, kind="ExternalInput").ap()
    woT = nc.dram_tensor("woT", [E, C], f16, kind="ExternalInput").ap()
    tok = nc.dram_tensor("tok", [1, 8], f32, kind="ExternalInput").ap()
    outT = nc.dram_tensor("outT", [C, T], f16, kind="ExternalOutput").ap()
    otok = nc.dram_tensor("otok", [1, 8], f32, kind="ExternalOutput").ap()

    NC_T = C // 128   # 8 c-tiles
    NST = T // 128    # 16 s-tiles
    LAG = 32          # PV runs LAG steps behind scores/exp

    with tile.TileContext(nc) as tc:
      for _rep in range(reps):
        with tc.tile_pool(name="persist", bufs=1) as persist, \
             tc.tile_pool(name="mps", bufs=1, space="PSUM") as mps, \
             tc.tile_pool(name="work", bufs=1) as work:
            tokt = persist.tile([1, 8], f32, tag="tokt", name="tokt")
            nc.sync.dma_start(tokt[:], tok)
            nc.sync.dma_start(otok, tokt[:])

            # persistent SBUF tensors
            qk = [persist.tile([128, T], f16, tag=f"qk{i}", name=f"qk{i}")
                  for i in range(8)]          # 0-3: q pairs, 4-7: k pairs
            vt = [persist.tile([128, HLOC * 65], f16, tag=f"v{i}", name=f"v{i}")
                  for i in range(NST)]
            onT = [persist.tile([128, T], f16, tag=f"on{i}", name=f"on{i}")
                   for i in range(4)]         # normalized attn out, [e,t]
            xs2 = [persist.tile([128, 2, T], f16, tag=f"x{i}", name=f"x{i}")
                   for i in range(NC_T // 2)]
            wqk2 = [persist.tile([128, 2, 2 * E], f16, tag=f"wqk{i}",
                    name=f"wqk{i}") for i in range(NC_T // 2)]
            xs = [xs2[i // 2][:, i % 2] for i in range(NC_T)]
            wqk = [wqk2[i // 2][:, i % 2] for i in range(NC_T)]
            wv = [persist.tile([128, E], f16, tag=f"wv{i}", name=f"wv{i}")
                  for i in range(NC_T)]
            wo = [persist.tile([128, C], f16, tag=f"wo{i}", name=f"wo{i}")
                  for i in range(4)]

            # Input DMAs.  HWDGE issue is a serial ~625ns/DMA resource, so
            # the SP stream carries only what gates the first scores (x and
            # the wqk columns holding q0/k0), x split in column halves so
            # pair-0 score production can start ~7us earlier.  Everything
            # else (wv, the rest of wqk, wo) rides the software DGE from the
            # otherwise-idle gpsimd/Pool engine, bypassing HWDGE.
            xT3 = xT.rearrange("(c p) t -> c p t", p=128)
            wqkT3 = wqkT.rearrange("(c p) t -> c p t", p=128)
            for cj in range(NC_T // 2):
                nc.sync.dma_start(
                    xs2[cj][:, :, 0:1024],
                    xT3[2 * cj:2 * cj + 2, :, 0:1024].rearrange(
                        "c p t -> p c t"))
                nc.sync.dma_start(
                    wqk2[cj][:, :, 0:640],
                    wqkT3[2 * cj:2 * cj + 2, :, 0:640].rearrange(
                        "c p t -> p c t"))
            for cj in range(NC_T // 2):
                nc.sync.dma_start(
                    xs2[cj][:, :, 1024:2048],
                    xT3[2 * cj:2 * cj + 2, :, 1024:2048].rearrange(
                        "c p t -> p c t"))
            for ci in range(NC_T):
                nc.sync.dma_start(wv[ci][:], wvT[ci * 128:(ci + 1) * 128, :])
            for cj in range(NC_T // 2):
                nc.gpsimd.dma_start(
                    wqk2[cj][:, :, 640:1024],
                    wqkT3[2 * cj:2 * cj + 2, :, 640:1024].rearrange(
                        "c p t -> p c t"))
            for i in range(4):
                nc.gpsimd.dma_start(wo[i][:], woT[i * 128:(i + 1) * 128, :])

            # ---------- emission helpers ----------
            def qk_chain(t8, col, nb):
                """One [e=128, t=512] block of the Q/K projection.
                t8: index into qk[]; col: e-tile column in wqkT."""
                ps = mps.tile([128, 512], f32, tag="proj", bufs=2, name="ps")
                for ci in range(NC_T):
                    nc.tensor.matmul(
                        ps[:],
                        lhsT=wqk[ci][:, col * 128:(col + 1) * 128],
                        rhs=xs[ci][:, nb * 512:(nb + 1) * 512],
                        start=(ci == 0), stop=(ci == NC_T - 1))
                nc.vector.tensor_copy(qk[t8][:, nb * 512:(nb + 1) * 512], ps[:])

            def v_chain(j):
                """V projection tile j: [t=128, e=512] + ones column."""
                ps = mps.tile([128, 512], f32, tag="proj", bufs=2, name="ps")
                for ci in range(NC_T):
                    nc.tensor.matmul(
                        ps[:],
                        lhsT=xs[ci][:, j * 128:(j + 1) * 128],
                        rhs=wv[ci][:],
                        start=(ci == 0), stop=(ci == NC_T - 1))
                v3 = vt[j][:].rearrange("p (h a) -> p h a", a=65)
                nc.vector.tensor_copy(
                    v3[:, :, 0:64],
                    ps[:].rearrange("p (h a) -> p h a", a=64))
                nc.gpsimd.memset(v3[:, :, 64:65], 1.0)

            def po_unit(tb, ft, out_sp=False):
                """Output projection: [c=128, t=512] block."""
                tsl = slice(tb * 512, (tb + 1) * 512)
                po = mps.tile([128, 512], f32, tag="proj", bufs=2, name="po")
                for p in range(4):
                    nc.tensor.matmul(
                        po[:],
                        lhsT=wo[p][:, ft * 128:(ft + 1) * 128],
                        rhs=onT[p][:, tsl],
                        start=(p == 0), stop=(p == 3))
                ot = work.tile([128, 512], f16, tag="ot", bufs=4, name="ot")
                nc.vector.tensor_copy(ot[:], po[:])
                # out-DMA via gpsimd SWDGE while scores still run (its
                # wait-for-copy parks on the idle Pool sequencer, never
                # stalling the SP DMA stream); the tail blocks go via SP
                # (idle by then, and HWDGE issue is faster than SWDGE gen)
                eng = nc.sync if out_sp else nc.gpsimd
                eng.dma_start(outT[ft * 128:(ft + 1) * 128, tsl], ot[:])

            # ---------- static filler schedule ----------
            # Just-in-time chain placement: scores(p,tb,st) reads k(p) block
            # st//4 and q(p) block tb, so each pair's k-chains go shortly
            # before its window and its q-chains one-per-tb *inside* it.
            # This spreads projection work evenly so the exp stream (the
            # serially-bound Activation engine) is never production-starved.
            from collections import defaultdict
            fill = defaultdict(list)
            # rest of k0 (scores st>=4m need block m) and q0 per-tb.
            # Order early fills by DMA arrival (xs-half0+wqk06, then wv,
            # then xs-half1) so a not-yet-landed transfer never head-of-line
            # blocks score production: k0nb1 first, V(0..7) after wv,
            # k0nb2/3 + V(8..15) after the second x half.
            fill[0].append(lambda: qk_chain(4, 4, 1))
            fill[6].append(lambda: qk_chain(4, 4, 2))
            for j in range(8):
                fill[8 + 2 * j].append(lambda j=j: v_chain(j))
            fill[10].append(lambda: qk_chain(0, 0, 1))
            fill[11].append(lambda: qk_chain(4, 4, 3))
            for j in range(8, NST):
                fill[16 + 4 * (j - 8)].append(lambda j=j: v_chain(j))
            fill[26].append(lambda: qk_chain(0, 0, 2))
            fill[42].append(lambda: qk_chain(0, 0, 3))
            # pairs 1..3, just-in-time: only k(nb0)+q(nb0) precede the
            # window (scores st<4 / tb0 need them); k(m) rides in-window
            # right before its first consumer st=4m, q(nb) before tb=nb.
            # This keeps each pair's window from being oversubscribed by
            # the next pair's projection chains.
            for p in range(1, 4):
                fill[64 * p - 5].append(lambda p=p: qk_chain(4 + p, 4 + p, 0))
                fill[64 * p - 3].append(lambda p=p: qk_chain(p, p, 0))
                for m in (1, 2, 3):
                    fill[64 * p + 4 * m - 3].append(
                        lambda p=p, m=m: qk_chain(4 + p, 4 + p, m))
                for nb in (1, 2, 3):
                    fill[64 * p + 16 * nb - 6].append(
                        lambda p=p, nb=nb: qk_chain(p, p, nb))
            # out-projection: tb0/tb1 interleave into the tail of pair 3
            # (a few steps after each normalize: the DVE-normalize ->
            # SP-transpose chain needs the latency before onT[3] is
            # readable); tb2/tb3 drain after the loop -- emitting them
            # earlier head-of-line blocks the PE on the transpose chain.
            for k in range(8):
                fill[244 + 2 * k].append(lambda k=k: po_unit(0, k))

            # ---------- main interleaved stream ----------
            pending = {}
            cur_aug = [None, None]

            def pv_step(g2):
                pt, p, tb, st = pending.pop(g2)
                if st == 0:
                    cur_aug[0] = mps.tile([128, 260], f32, tag="augA",
                                          bufs=1, name="augA")
                    cur_aug[1] = mps.tile([128, 260], f32, tag="augB",
                                          bufs=1, name="augB")
                augA, augB = cur_aug
                hA, hB = 2 * p, 2 * p + 1
                # One PSUM accumulation group per aug bank: start marks the
                # whole 2KB zero-region pending-zero, so only (st=0,u=0)
                # starts; the u=1..3 first writes at st=0 hit pending-zero
                # bytes and initialize; stop only on the last matmul.
                for u in range(4):
                    nc.tensor.matmul(
                        augA[:, u * 65:(u + 1) * 65],
                        lhsT=pt[:, u * 128:(u + 1) * 128],
                        rhs=vt[st][:, hA * 65:(hA + 1) * 65],
                        start=(st == 0 and u == 0),
                        stop=(st == NST - 1 and u == 3))
                    nc.tensor.matmul(
                        augB[:, u * 65:(u + 1) * 65],
                        lhsT=pt[:, 512 + u * 128:512 + (u + 1) * 128],
                        rhs=vt[st][:, hB * 65:(hB + 1) * 65],
                        start=(st == 0 and u == 0),
                        stop=(st == NST - 1 and u == 3))
                if st == NST - 1:
                    normalize(p, tb, augA, augB)

            def normalize(p, tb, augA, augB):
                a3 = augA[:].rearrange("p (u a) -> p u a", a=65)
                b3 = augB[:].rearrange("p (u a) -> p u a", a=65)
                rcA = work.tile([128, 4], f32, tag="rc", bufs=4, name="rcA")
                rcB = work.tile([128, 4], f32, tag="rc", bufs=4, name="rcB")
                nc.vector.reciprocal(rcA[:], a3[:, :, 64])
                nc.vector.reciprocal(rcB[:], b3[:, :, 64])
                for u in range(4):
                    ntp = work.tile([128, 128], f16, tag="ntp", bufs=4,
                                    name="ntp")
                    nc.vector.tensor_scalar_mul(
                        ntp[:, 0:64], a3[:, u, 0:64], rcA[:, u:u + 1])
                    nc.vector.tensor_scalar_mul(
                        ntp[:, 64:128], b3[:, u, 0:64], rcB[:, u:u + 1])
                    nc.sync.dma_start_transpose(
                        onT[p][:, tb * 512 + u * 128:tb * 512 + (u + 1) * 128],
                        ntp[:])

            # head: k0/q0 block 0 only -- first 4 score s-tiles (and tb0)
            # need just these, so scoring starts as soon as the first x
            # column-half lands
            qk_chain(4, 4, 0)
            qk_chain(0, 0, 0)

            for g in range(256 + LAG + 16):
                if g < 256:
                    p, tb, st = g // 64, (g // 16) % 4, g % 16
                    qt, kt = qk[p], qk[4 + p]
                    tsl = slice(tb * 512, (tb + 1) * 512)
                    ssl = slice(st * 128, (st + 1) * 128)
                    pss = mps.tile([128, 1024], f32, tag="pss", bufs=2,
                                   name="pss")
                    nc.tensor.matmul(
                        pss[:, 0:512],
                        lhsT=kt[0:64, ssl], rhs=qt[0:64, tsl],
                        start=True, stop=True, tile_position=(0, 0))
                    nc.tensor.matmul(
                        pss[:, 512:1024],
                        lhsT=kt[64:128, ssl], rhs=qt[64:128, tsl],
                        start=True, stop=True, tile_position=(64, 0))
                    pt = work.tile([128, 1024], f16, tag="P", bufs=36,
                                   name="pt")
                    nc.scalar.activation(pt[:], pss[:], EXP, scale=0.125)
                    pending[g] = (pt, p, tb, st)
                if LAG <= g < 256 + LAG:
                    pv_step(g - LAG)
                for cl in fill.get(g, []):
                    cl()
            # tb2/tb3 drain after the loop: normalize(p3,tb3) then lands in
            # the DVE queue ahead of these units' copies, so the last
            # transposes aren't delayed behind them
            for ft in range(8):
                po_unit(1, ft, out_sp=True)
            for ft in range(8):
                po_unit(2, ft, out_sp=True)
            for ft in range(8):
                po_unit(3, ft, out_sp=True)

    nc.compile()
    return nc


def _get_nc():
    if "nc" not in _CACHE:
        _CACHE["nc"] = _build()
    return _CACHE["nc"]


def _shard(x, w_qkv, w_out):
    in_maps = []
    for c in range(NCORES):
        b, g = divmod(c, 2)
        gs = slice(g * E, (g + 1) * E)
        wq = w_qkv[0 * C:1 * C][gs]            # [512, 1024]
        wk = w_qkv[1 * C:2 * C][gs]
        wv = w_qkv[2 * C:3 * C][gs]
        in_maps.append({
            "xT": np.ascontiguousarray(x[b].T).astype(np.float16),
            "wqkT": np.ascontiguousarray(
                np.concatenate([wq, wk], 0).T).astype(np.float16),
            "wvT": np.ascontiguousarray(wv.T).astype(np.float16),
            "woT": np.ascontiguousarray(w_out[:, gs].T).astype(np.float16),
            "tok": np.zeros((1, 8), dtype=np.float32),
        })
    return in_maps


def kernel(x, w_qkv, w_out, b_out, _trace=False):
    from concourse.bass_utils import run_bass_kernel_spmd

    x = np.asarray(x, dtype=np.float32)
    w_qkv = np.asarray(w_qkv, dtype=np.float32)
    w_out = np.asarray(w_out, dtype=np.float32)
    b_out = np.asarray(b_out, dtype=np.float32)

    nc = _get_nc()
    in_maps = _shard(x, w_qkv, w_out)
    res = run_bass_kernel_spmd(nc, in_maps, list(range(NCORES)), trace=_trace)
    _CACHE["last_result"] = res

    out = np.empty((B, T, C), dtype=np.float32)
    for b in range(B):
        acc = (res.results[2 * b]["outT"].astype(np.float32)
               + res.results[2 * b + 1]["outT"].astype(np.float32))
        out[b] = acc.T + b_out
    return out


# revision 48
# speedup vs baseline: 1.0055x; 1.0055x over previous
"""Multi-head attention (B=4, T=2048, dim=1024, 16 heads) on 8 NeuronCores.

Sharding: core c -> batch b = c//2, head-group g = c%2 (8 heads per core).
Each core computes QKV projection for its heads, full attention over its
8 heads, and a partial output projection (contraction over its 512 head
dims). Host sums the two fp16 partials per batch and adds b_out.

v2 design (fp16 everywhere, ACT/PE co-scheduled):
  All matmul operands fp16 (PE streams 1 col/cycle, same as f32r, but
  SBUF halves so the full T=2048 stays resident; PSUM accumulation fp32).
  Attention uses a "flipped" PV matmul: P^T tiles ([s,t-chunk]) are the
  stationary operand against the moving V_aug [s, 65] (65th col = ones
  for the softmax denominator), producing out_aug [t, 65] -- 520 streamed
  columns per (p,tb,st) instead of 1024.  Normalization is then a cheap
  per-partition broadcast (DVE tensor_scalar with an AP scalar), and the
  [t,e] -> [e,t] layout fix rides the XBAR DMA-transpose (free wrt PE).

  The single instruction stream interleaves score production (which feeds
  the serially-bound Activation engine: 256 exp tiles x ~1.04us = 266us)
  with V-projection / next-pair QK-projection / output-projection filler
  so the PE never idles: PV lags scores by 16 steps (pt pool bufs=20)
  which also lets the V projection stream in during pair 0's window.

  PSUM budget (8 banks): pss [128,1024]x2 (4) + proj [128,512]x2 (2) +
  augA/augB [128,260]x1 (2).
"""

import numpy as np

B, T, C = 4, 2048, 1024
HLOC = 8          # heads per core
D = 64
E = HLOC * D      # 512: local head-dim width
NCORES = 8

_CACHE = {}


def _build(reps=1):
    import concourse.mybir as mybir
    import concourse.tile as tile
    import concourse.bacc as bacc

    f32 = mybir.dt.float32
    f16 = mybir.dt.float16
    EXP = mybir.ActivationFunctionType.Exp

    nc = bacc.Bacc("TRN2", target_bir_lowering=False, debug=False)

    xT = nc.dram_tensor("xT", [C, T], f16, kind="ExternalInput").ap()
    wqkT = nc.dram_tensor("wqkT", [C, 2 * E], f16, kind="ExternalInput").ap()
    wvT = nc.dram_tensor("wvT", [C, E], f16, kind="ExternalInput").ap()
    woT = nc.dram_tensor("woT", [E, C], f16, kind="ExternalInput").ap()
    tok = nc.dram_tensor("tok", [1, 8], f32, kind="ExternalInput").ap()
    outT = nc.dram_tensor("outT", [C, T], f16, kind="ExternalOutput").ap()
    otok = nc.dram_tensor("otok", [1, 8], f32, kind="ExternalOutput").ap()

    NC_T = C // 128   # 8 c-tiles
    NST = T // 128    # 16 s-tiles
    LAG = 36          # PV runs LAG steps behind scores/exp

    with tile.TileContext(nc) as tc:
      for _rep in range(reps):
        with tc.tile_pool(name="persist", bufs=1) as persist, \
             tc.tile_pool(name="mps", bufs=1, space="PSUM") as mps, \
             tc.tile_pool(name="work", bufs=1) as work:
            tokt = persist.tile([1, 8], f32, tag="tokt", name="tokt")
            nc.sync.dma_start(tokt[:], tok)
            nc.sync.dma_start(otok, tokt[:])

            # persistent SBUF tensors
            qk = [persist.tile([128, T], f16, tag=f"qk{i}", name=f"qk{i}")
                  for i in range(8)]          # 0-3: q pairs, 4-7: k pairs
            vt = [persist.tile([128, HLOC * 65], f16, tag=f"v{i}", name=f"v{i}")
                  for i in range(NST)]
            # normalized attn out [e,t]; pairs 1-3 overlay the spent q
            # tiles (q[p-1] is dead before onT[p]'s first transpose lands)
            onT0 = persist.tile([128, T], f16, tag="on0", name="onT0")
            onT = [onT0, qk[0], qk[1], qk[2]]
            xs2 = [persist.tile([128, 2, T], f16, tag=f"x{i}", name=f"x{i}")
                   for i in range(NC_T // 2)]
            wqk2 = [persist.tile([128, 2, 2 * E], f16, tag=f"wqk{i}",
                    name=f"wqk{i}") for i in range(NC_T // 2)]
            xs = [xs2[i // 2][:, i % 2] for i in range(NC_T)]
            wqk = [wqk2[i // 2][:, i % 2] for i in range(NC_T)]
            wv = [persist.tile([128, E], f16, tag=f"wv{i}", name=f"wv{i}")
                  for i in range(NC_T)]
            wo = [persist.tile([128, C], f16, tag=f"wo{i}", name=f"wo{i}")
                  for i in range(4)]

            # Input DMAs.  HWDGE issue is a serial ~625ns/DMA resource, so
            # the SP stream carries only what gates the first scores (x and
            # the wqk columns holding q0/k0), x split in column halves so
            # pair-0 score production can start ~7us earlier.  Everything
            # else (wv, the rest of wqk, wo) rides the software DGE from the
            # otherwise-idle gpsimd/Pool engine, bypassing HWDGE.
            xT3 = xT.rearrange("(c p) t -> c p t", p=128)
            wqkT3 = wqkT.rearrange("(c p) t -> c p t", p=128)
            for cj in range(NC_T // 2):
                nc.sync.dma_start(
                    xs2[cj][:, :, 0:1024],
                    xT3[2 * cj:2 * cj + 2, :, 0:1024].rearrange(
                        "c p t -> p c t"))
                nc.sync.dma_start(
                    wqk2[cj][:, :, 0:640],
                    wqkT3[2 * cj:2 * cj + 2, :, 0:640].rearrange(
                        "c p t -> p c t"))
            for cj in range(NC_T // 2):
                nc.sync.dma_start(
                    xs2[cj][:, :, 1024:2048],
                    xT3[2 * cj:2 * cj + 2, :, 1024:2048].rearrange(
                        "c p t -> p c t"))
            for ci in range(NC_T):
                nc.sync.dma_start(wv[ci][:], wvT[ci * 128:(ci + 1) * 128, :])
            for cj in range(NC_T // 2):
                nc.gpsimd.dma_start(
                    wqk2[cj][:, :, 640:1024],
                    wqkT3[2 * cj:2 * cj + 2, :, 640:1024].rearrange(
                        "c p t -> p c t"))
            for i in range(4):
                nc.gpsimd.dma_start(wo[i][:], woT[i * 128:(i + 1) * 128, :])

            # ---------- emission helpers ----------
            def qk_chain(t8, col, nb):
                """One [e=128, t=512] block of the Q/K projection.
                t8: index into qk[]; col: e-tile column in wqkT."""
                ps = mps.tile([128, 512], f32, tag="proj", bufs=2, name="ps")
                for ci in range(NC_T):
                    nc.tensor.matmul(
                        ps[:],
                        lhsT=wqk[ci][:, col * 128:(col + 1) * 128],
                        rhs=xs[ci][:, nb * 512:(nb + 1) * 512],
                        start=(ci == 0), stop=(ci == NC_T - 1))
                nc.vector.tensor_copy(qk[t8][:, nb * 512:(nb + 1) * 512], ps[:])

            def v_chain(j):
                """V projection tile j: [t=128, e=512] + ones column."""
                ps = mps.tile([128, 512], f32, tag="proj", bufs=2, name="ps")
                for ci in range(NC_T):
                    nc.tensor.matmul(
                        ps[:],
                        lhsT=xs[ci][:, j * 128:(j + 1) * 128],
                        rhs=wv[ci][:],
                        start=(ci == 0), stop=(ci == NC_T - 1))
                v3 = vt[j][:].rearrange("p (h a) -> p h a", a=65)
                nc.vector.tensor_copy(
                    v3[:, :, 0:64],
                    ps[:].rearrange("p (h a) -> p h a", a=64))
                nc.gpsimd.memset(v3[:, :, 64:65], 1.0)

            def po_unit(tb, ft, out_sp=False):
                """Output projection: [c=128, t=512] block."""
                tsl = slice(tb * 512, (tb + 1) * 512)
                po = mps.tile([128, 512], f32, tag="proj", bufs=2, name="po")
                for p in range(4):
                    nc.tensor.matmul(
                        po[:],
                        lhsT=wo[p][:, ft * 128:(ft + 1) * 128],
                        rhs=onT[p][:, tsl],
                        start=(p == 0), stop=(p == 3))
                ot = work.tile([128, 512], f16, tag="ot", bufs=4, name="ot")
                nc.vector.tensor_copy(ot[:], po[:])
                # out-DMA via gpsimd SWDGE while scores still run (its
                # wait-for-copy parks on the idle Pool sequencer, never
                # stalling the SP DMA stream); the tail blocks go via SP
                # (idle by then, and HWDGE issue is faster than SWDGE gen)
                eng = nc.sync if out_sp else nc.gpsimd
                eng.dma_start(outT[ft * 128:(ft + 1) * 128, tsl], ot[:])

            # ---------- static filler schedule ----------
            # Just-in-time chain placement: scores(p,tb,st) reads k(p) block
            # st//4 and q(p) block tb, so each pair's k-chains go shortly
            # before its window and its q-chains one-per-tb *inside* it.
            # This spreads projection work evenly so the exp stream (the
            # serially-bound Activation engine) is never production-starved.
            from collections import defaultdict
            fill = defaultdict(list)
            # rest of k0 (scores st>=4m need block m) and q0 per-tb.
            # Order early fills by DMA arrival (xs-half0+wqk06, then wv,
            # then xs-half1) so a not-yet-landed transfer never head-of-line
            # blocks score production: k0nb1 first, V(0..7) after wv,
            # k0nb2/3 + V(8..15) after the second x half.
            fill[0].append(lambda: qk_chain(4, 4, 1))
            fill[6].append(lambda: qk_chain(4, 4, 2))
            for j in range(8):
                fill[8 + j].append(lambda j=j: v_chain(j))
            fill[10].append(lambda: qk_chain(0, 0, 1))
            fill[11].append(lambda: qk_chain(4, 4, 3))
            for j in range(8, NST):
                fill[16 + 4 * (j - 8)].append(lambda j=j: v_chain(j))
            fill[26].append(lambda: qk_chain(0, 0, 2))
            fill[42].append(lambda: qk_chain(0, 0, 3))
            # pairs 1..3, just-in-time: only k(nb0)+q(nb0) precede the
            # window (scores st<4 / tb0 need them); k(m) rides in-window
            # right before its first consumer st=4m, q(nb) before tb=nb.
            # This keeps each pair's window from being oversubscribed by
            # the next pair's projection chains.
            for p in range(1, 4):
                fill[64 * p - 5].append(lambda p=p: qk_chain(4 + p, 4 + p, 0))
                fill[64 * p - 3].append(lambda p=p: qk_chain(p, p, 0))
                for m in (1, 2, 3):
                    fill[64 * p + 4 * m - 3].append(
                        lambda p=p, m=m: qk_chain(4 + p, 4 + p, m))
                for nb in (1, 2, 3):
                    fill[64 * p + 16 * nb - 6].append(
                        lambda p=p, nb=nb: qk_chain(p, p, nb))
            # out-projection: tb0/tb1 interleave into the tail of pair 3
            # (a few steps after each normalize: the DVE-normalize ->
            # SP-transpose chain needs the latency before onT[3] is
            # readable); tb2/tb3 drain after the loop -- emitting them
            # earlier head-of-line blocks the PE on the transpose chain.
            for k in range(8):
                fill[248 + k].append(lambda k=k: po_unit(0, k))

            # ---------- main interleaved stream ----------
            pending = {}
            cur_aug = [None, None]

            def pv_step(g2):
                pt, p, tb, st = pending.pop(g2)
                if st == 0:
                    cur_aug[0] = mps.tile([128, 260], f32, tag="augA",
                                          bufs=1, name="augA")
                    cur_aug[1] = mps.tile([128, 260], f32, tag="augB",
                                          bufs=1, name="augB")
                augA, augB = cur_aug
                hA, hB = 2 * p, 2 * p + 1
                # One PSUM accumulation group per aug bank: start marks the
                # whole 2KB zero-region pending-zero, so only (st=0,u=0)
                # starts; the u=1..3 first writes at st=0 hit pending-zero
                # bytes and initialize; stop only on the last matmul.
                for u in range(4):
                    nc.tensor.matmul(
                        augA[:, u * 65:(u + 1) * 65],
                        lhsT=pt[:, u * 128:(u + 1) * 128],
                        rhs=vt[st][:, hA * 65:(hA + 1) * 65],
                        start=(st == 0 and u == 0),
                        stop=(st == NST - 1 and u == 3))
                    nc.tensor.matmul(
                        augB[:, u * 65:(u + 1) * 65],
                        lhsT=pt[:, 512 + u * 128:512 + (u + 1) * 128],
                        rhs=vt[st][:, hB * 65:(hB + 1) * 65],
                        start=(st == 0 and u == 0),
                        stop=(st == NST - 1 and u == 3))
                if st == NST - 1:
                    normalize(p, tb, augA, augB)

            def normalize(p, tb, augA, augB):
                a3 = augA[:].rearrange("p (u a) -> p u a", a=65)
                b3 = augB[:].rearrange("p (u a) -> p u a", a=65)
                rcA = work.tile([128, 4], f32, tag="rc", bufs=4, name="rcA")
                rcB = work.tile([128, 4], f32, tag="rc", bufs=4, name="rcB")
                nc.vector.reciprocal(rcA[:], a3[:, :, 64])
                nc.vector.reciprocal(rcB[:], b3[:, :, 64])
                for u in range(4):
                    ntp = work.tile([128, 128], f16, tag="ntp", bufs=4,
                                    name="ntp")
                    nc.vector.tensor_scalar_mul(
                        ntp[:, 0:64], a3[:, u, 0:64], rcA[:, u:u + 1])
                    nc.vector.tensor_scalar_mul(
                        ntp[:, 64:128], b3[:, u, 0:64], rcB[:, u:u + 1])
                    nc.sync.dma_start_transpose(
                        onT[p][:, tb * 512 + u * 128:tb * 512 + (u + 1) * 128],
                        ntp[:])

            # head: k0/q0 block 0 only -- first 4 score s-tiles (and tb0)
            # need just these, so scoring starts as soon as the first x
            # column-half lands
            qk_chain(4, 4, 0)
            qk_chain(0, 0, 0)

            for g in range(256 + LAG + 16):
                if g < 256:
                    p, tb, st = g // 64, (g // 16) % 4, g % 16
                    qt, kt = qk[p], qk[4 + p]
                    tsl = slice(tb * 512, (tb + 1) * 512)
                    ssl = slice(st * 128, (st + 1) * 128)
                    pss = mps.tile([128, 1024], f32, tag="pss", bufs=2,
                                   name="pss")
                    nc.tensor.matmul(
                        pss[:, 0:512],
                        lhsT=kt[0:64, ssl], rhs=qt[0:64, tsl],
                        start=True, stop=True, tile_position=(0, 0))
                    nc.tensor.matmul(
                        pss[:, 512:1024],
                        lhsT=kt[64:128, ssl], rhs=qt[64:128, tsl],
                        start=True, stop=True, tile_position=(64, 0))
                    pt = work.tile([128, 1024], f16, tag="P", bufs=40,
                                   name="pt")
                    nc.scalar.activation(pt[:], pss[:], EXP, scale=0.125)
                    pending[g] = (pt, p, tb, st)
                if LAG <= g < 256 + LAG:
                    pv_step(g - LAG)
                for cl in fill.get(g, []):
                    cl()
            # tb2/tb3 drain after the loop: normalize(p3,tb3) then lands in
            # the DVE queue ahead of these units' copies, so the last
            # transposes aren't delayed behind them
            for ft in range(8):
                po_unit(1, ft, out_sp=True)
            for ft in range(8):
                po_unit(2, ft, out_sp=True)
            for ft in range(8):
                po_unit(3, ft, out_sp=True)

    nc.compile()
    return nc


def _get_nc():
    if "nc" not in _CACHE:
        _CACHE["nc"] = _build()
    return _CACHE["nc"]


def _shard(x, w_qkv, w_out):
    in_maps = []
    for c in range(NCORES):
        b, g = divmod(c, 2)
        gs = slice(g * E, (g + 1) * E)
        wq = w_qkv[0 * C:1 * C][gs]            # [512, 1024]
        wk = w_qkv[1 * C:2 * C][gs]
        wv = w_qkv[2 * C:3 * C][gs]
        in_maps.append({
            "xT": np.ascontiguousarray(x[b].T).astype(np.float16),
            "wqkT": np.ascontiguousarray(
                np.concatenate([wq, wk], 0).T).astype(np.float16),
            "wvT": np.ascontiguousarray(wv.T).astype(np.float16),
            "woT": np.ascontiguousarray(w_out[:, gs].T).astype(np.float16),
            "tok": np.zeros((1, 8), dtype=np.float32),
        })
    return in_maps


def kernel(x, w_qkv, w_out, b_out, _trace=False):
    from concourse.bass_utils import run_bass_kernel_spmd

    x = np.asarray(x, dtype=np.float32)
    w_qkv = np.asarray(w_qkv, dtype=np.float32)
    w_out = np.asarray(w_out, dtype=np.float32)
    b_out = np.asarray(b_out, dtype=np.float32)

    nc = _get_nc()
    in_maps = _shard(x, w_qkv, w_out)
    res = run_bass_kernel_spmd(nc, in_maps, list(range(NCORES)), trace=_trace)
    _CACHE["last_result"] = res

    out = np.empty((B, T, C), dtype=np.float32)
    for b in range(B):
        acc = (res.results[2 * b]["outT"].astype(np.float32)
               + res.results[2 * b + 1]["outT"].astype(np.float32))
        out[b] = acc.T + b_out
    return out
